# revision 16
# baseline (speedup 1.0000x reference)
"""Trainium2 Bass kernel for nn_Attention_Module (sparse_attention).

Computation per batch b (x_b: [C=256, T=4096] fp32):
    energy = x_b @ x_b^T                      # (256, 256), K=4096
    attn   = softmax(rowmax(energy) - energy) # == exp(mu - e)/Z, mu = rowmin
    out    = gamma * (attn @ x_b) + x_b

Strategy (8 cores, pure data-parallel, 4 batches/core):
  - x is loaded from HBM exactly ONCE, as fp16 in t-major layout
    (xt[b, p, k, c] = x[b, c, k*128+p]) with 8KB/partition DMA lines
    (1 descriptor per partition -> cheap HWDGE issue). The c-major copy
    needed by matmul2 (xn) is derived on-chip with PE transposes.
    DMA drops from 41.9 MB/core (baseline) to 25.2 MB/core.
  - Energy is symmetric: only blocks 00/01/11 are computed; block 10 is
    a PE transpose of block 01 (saves 1/4 of matmul1).
  - matmul1 (both row-block chains) and the xn transposes are
    interleaved per k-tile so the PE tracks DMA arrival during the
    pipeline fill instead of stalling on whole tiles.
  - B = gamma*diag(1/Z)*P + I is materialized directly as the matmul2
    weights: P rows are scaled by 1/Z before the PE transpose, gamma is
    applied during the PSUM->SBUF copy of P^T, and the +x residual is
    the exact identity diagonal. matmul2 yields final output values, so
    PSUM->SBUF drains are plain copies (round-robined ACT/DVE).
  - Software pipeline: slot b = loads(b+1), mm1+xnT(b), mm2(b-1)
    (hides b's softmax latency), then softmax/At(b).
"""

import numpy as np

B, C, T = 32, 256, 4096
NCORES = 8
NB = B // NCORES  # batches per core
P = 128
KT = T // P  # 32 t-tiles of 128
KH = KT // 2  # 16 per half (xn is built as two half-tiles)
TC = T // 512  # 8 output chunks per m-block

_CACHE = {}


def _build_nc(variant=None):
    variant = variant or {}
    from contextlib import ExitStack

    import concourse.bacc as bacc
    import concourse.bass as bass
    import concourse.tile as tile
    from concourse import mybir

    f32 = mybir.dt.float32
    f16 = mybir.dt.float16
    ts = bass.ts

    nc = bacc.Bacc(
        "TRN2",
        target_bir_lowering=False,
        debug=False,
        enable_asserts=False,
        num_devices=NCORES,
    )

    # xt[b, p, k, c] = x[b, c, k*128+p]; per-partition lines are 8KB.
    xt_h = nc.dram_tensor("xt", [NB, P, KT, C], f16, kind="ExternalInput")
    # xnh[b, k2, p, t'] = x[b, k2*128+p, 2048+t']: c-major upper half of t,
    # loaded directly so only half of xn needs PE transposes.
    xnh_h = nc.dram_tensor("xnh", [NB, 2, P, T // 2], f16, kind="ExternalInput")
    # aux: col0 = gamma (f32), cols 4:132 = identity (f32)
    aux_h = nc.dram_tensor("aux", [P, 132], f32, kind="ExternalInput")
    idn_h = nc.dram_tensor("idn", [P, P], f16, kind="ExternalInput")
    o_h = nc.dram_tensor("o", [NB, C, T], f32, kind="ExternalOutput")

    with tile.TileContext(nc) as tc:
        with ExitStack() as ctx:
            singles = ctx.enter_context(tc.tile_pool(name="singles", bufs=1))
            xt_pool = ctx.enter_context(tc.tile_pool(name="xt", bufs=2))
            xn_pool = ctx.enter_context(tc.tile_pool(name="xn", bufs=3))
            out_pool = ctx.enter_context(tc.tile_pool(name="out", bufs=3))
            att_pool = ctx.enter_context(tc.tile_pool(name="att", bufs=2))
            small = ctx.enter_context(tc.tile_pool(name="small", bufs=3))
            psum_e = ctx.enter_context(
                tc.tile_pool(name="psum_e", bufs=1, space="PSUM")
            )
            psum_x = ctx.enter_context(
                tc.tile_pool(name="psum_x", bufs=2, space="PSUM")
            )
            psum_o = ctx.enter_context(
                tc.tile_pool(name="psum_o", bufs=4, space="PSUM")
            )

            xt_ap = xt_h.ap()
            xnh_ap = xnh_h.ap()
            o_ap = o_h.ap()

            aux = singles.tile([P, 132], f32)
            nc.scalar.dma_start(aux[:], aux_h.ap())
            idn = singles.tile([P, P], f16)
            nc.scalar.dma_start(idn[:], idn_h.ap())
            gv = aux[:, 0:1]
            idn32 = aux[:, 4:132]

            def issue_loads(b):
                xna = xn_pool.tile(
                    [P, 2, T // 2], f16, tag="xna", name="xna"
                )
                xnb = xn_pool.tile(
                    [P, 2, T // 2], f16, tag="xnb", name="xnb"
                )
                if b == 0:
                    # four separate tiles on ACT (its DGE is up ~4us before
                    # SP's): mm1 starts after the first 512KB lands
                    KQ = KT // 4
                    qs = []
                    for q in range(4):
                        t_ = xt_pool.tile(
                            [P, KQ, C], f16, tag=f"xq{q}", name=f"xq{q}"
                        )
                        nc.scalar.dma_start(
                            t_[:], xt_ap[b, :, q * KQ : (q + 1) * KQ, :]
                        )
                        qs.append(t_)
                    nc.sync.dma_start(
                        xnb[:],
                        xnh_ap[b].rearrange("k p t -> p k t"),
                    )
                    return (qs, KT // 4, xna, xnb)
                xta = xt_pool.tile([P, KT, C], f16, tag="xta", name="xta")
                nc.sync.dma_start(xta[:], xt_ap[b])
                nc.sync.dma_start(
                    xnb[:], xnh_ap[b].rearrange("k p t -> p k t")
                )
                return ([xta], KT, xna, xnb)

            # round-robin copy engines for PSUM->SBUF drains
            cp_engines = [
                lambda o, i: nc.scalar.copy(o, i),
                lambda o, i: nc.vector.tensor_copy(o, i),
            ]
            cp_idx = [0]

            def copy_eng():
                e = cp_engines[cp_idx[0] % len(cp_engines)]
                cp_idx[0] += 1
                return e

            def run_mm2(pb, pAt, pxn):
                """out(pb) = B^T @ x (final values) + pipelined stores."""
                # store granularity: half-tiles (2KB lines); last batch uses
                # quarter-tiles so the final store wave is only 1MB deep
                nst = 4 if pb == NB - 1 else 2
                csz = TC // nst  # 512-chunks per store tile
                TH = T // 2
                orr = o_ap[pb].rearrange("(m p) t -> p m t", p=P)
                for m in range(2):
                    for s in range(nst):
                        ot = out_pool.tile(
                            [P, csz * 512], f32, tag=f"ot{nst}", name="ot"
                        )
                        for c in range(csz):
                            t8 = s * csz + c
                            po = psum_o.tile([P, 512], f32)
                            tf = t8 * 512 % TH
                            for k in range(2):
                                nc.tensor.matmul(
                                    po[:],
                                    lhsT=pAt[m][:, k, :],
                                    rhs=pxn[t8 // 4][:, k, tf : tf + 512],
                                    start=(k == 0),
                                    stop=(k == 1),
                                )
                            copy_eng()(ot[:, ts(c, 512)], po[:])
                        nc.sync.dma_start(
                            orr[:, m, s * csz * 512 : (s + 1) * csz * 512],
                            ot[:],
                        )

            tiles = {0: issue_loads(0)}
            pending = None  # (b, At, xn) awaiting matmul2

            for b in range(NB):
                xtiles, kdiv, xna, xnb = tiles.pop(b)
                if b + 1 < NB:
                    tiles[b + 1] = issue_loads(b + 1)

                xn = [xna, xnb]
                At = [
                    att_pool.tile([P, 2, P], f16, tag="Ata", name="Ata"),
                    att_pool.tile([P, 2, P], f16, tag="Atb", name="Atb"),
                ]
                Zs = small.tile([P, 2], f32, tag="Zs")
                rZ = small.tile([P, 2], f32, tag="rZ")

                # ---- interleaved mm1 (blocks 00/01 + block 11) and xn
                # transposes, per k-tile, tracking DMA arrival ----
                pe1 = psum_e.tile([P, C], f32, tag="pe1", name="pe1")
                pe2 = psum_e.tile([P, C], f32, tag="pe2", name="pe2")
                for k in range(KT):
                    src = xtiles[k // kdiv]
                    kk = k % kdiv
                    nc.tensor.matmul(
                        pe1[:],
                        lhsT=src[:, kk, ts(0, P)],
                        rhs=src[:, kk, :],
                        start=(k == 0),
                        stop=(k == KT - 1),
                    )
                    nc.tensor.matmul(
                        pe2[:, ts(1, P)],
                        lhsT=src[:, kk, ts(1, P)],
                        rhs=src[:, kk, ts(1, P)],
                        start=(k == 0),
                        stop=(k == KT - 1),
                    )
                    if k % 4 == 3 and k < KH:
                        g4 = k - 3
                        half = 0
                        tb = g4 * P
                        for cb in range(2):
                            px = psum_x.tile([P, 512], f16, tag="px", name="px")
                            for j in range(4):
                                kg = g4 + j
                                nc.tensor.transpose(
                                    px[:, ts(j, P)],
                                    xtiles[kg // kdiv][:, kg % kdiv, ts(cb, P)],
                                    idn[:],
                                )
                            copy_eng()(xn[half][:, cb, tb : tb + 512], px[:])

                # energy block 10 = (block 01)^T: stage 01 to SBUF, PE
                # transpose (f32) straight into pe2's first half.
                s01 = small.tile([P, P], f32, tag="s01")
                nc.scalar.copy(s01[:], pe1[:, ts(1, P)])

                # ---- previous batch's matmul2 here: hides this batch's
                # softmax/At latency behind PE work ----
                if pending is not None:
                    run_mm2(*pending)
                    pending = None

                nc.tensor.transpose(pe2[:, ts(0, P)], s01[:], idn32)

                for m in range(2):
                    pe = (pe1, pe2)[m]
                    mu = small.tile([P, 1], f32, tag="mu")
                    nc.vector.tensor_reduce(
                        mu[:], pe[:], axis=mybir.AxisListType.X,
                        op=mybir.AluOpType.min,
                    )
                    Pm = small.tile([P, C], f32, tag="Pm")
                    nc.scalar.activation(
                        Pm[:],
                        pe[:],
                        mybir.ActivationFunctionType.Exp,
                        bias=mu[:],
                        scale=-1.0,
                        accum_out=Zs[:, m : m + 1],
                    )
                    nc.vector.reciprocal(rZ[:, m : m + 1], Zs[:, m : m + 1])
                    Pm2 = small.tile([P, C], f16, tag="Pm2")
                    nc.vector.tensor_scalar_mul(Pm2[:], Pm[:], rZ[:, m : m + 1])

                    # At[m][:, k2, :] = gamma * (P/Z)^T (+ I on diagonal)
                    ptf = psum_x.tile([P, 512], f16, tag="px", name="ptf")
                    pt = ptf[:, :C]
                    for k2 in range(2):
                        nc.tensor.transpose(
                            pt[:, ts(k2, P)], Pm2[:, ts(k2, P)], idn[:]
                        )
                    nc.scalar.mul(At[m][:, :, :], pt[:], gv)
                    nc.vector.tensor_add(
                        At[m][:, m, :], At[m][:, m, :], idn[:]
                    )

                this = (b, At, xn)
                if b == NB - 1:
                    run_mm2(*this)
                else:
                    pending = this

    nc.compile()
    return nc


def _get_nc():
    if "nc" not in _CACHE:
        _CACHE["nc"] = _build_nc()
    return _CACHE["nc"]


def _make_aux(gamma_val):
    aux = np.zeros((P, 132), dtype=np.float32)
    aux[:, 0] = gamma_val
    aux[:, 4:132] = np.eye(P, dtype=np.float32)
    return aux


def kernel(x, gamma, _trace=False):
    import concourse.bass_utils as bass_utils

    x = np.ascontiguousarray(np.asarray(x, dtype=np.float32))
    gamma = np.asarray(gamma, dtype=np.float32).reshape(-1)

    nc = _get_nc()

    aux = _make_aux(gamma[0])
    idn = np.eye(P, dtype=np.float16)
    # xt[b, p, k, c] = x[b, c, k*128+p]
    xt_all = (
        x.astype(np.float16)
        .reshape(B, C, KT, P)
        .transpose(0, 3, 2, 1)
    )
    xnh_all = (
        x[:, :, T // 2 :].astype(np.float16).reshape(B, 2, P, T // 2)
    )
    in_maps = []
    for d in range(NCORES):
        in_maps.append(
            {
                "xt": np.ascontiguousarray(xt_all[d * NB : (d + 1) * NB]),
                "xnh": np.ascontiguousarray(xnh_all[d * NB : (d + 1) * NB]),
                "aux": aux,
                "idn": idn,
            }
        )

    res = bass_utils.run_bass_kernel_spmd(
        nc, in_maps, core_ids=list(range(NCORES)), trace=_trace
    )
    out = np.concatenate([r["o"] for r in res.results], axis=0)
    if _trace:
        _CACHE["last_results"] = res
    return out


# revision 17
# speedup vs baseline: 1.2056x; 1.2056x over previous
"""Trainium2 Bass kernel for nn_Attention_Module (sparse_attention).

Computation per batch b (x_b: [C=256, T=4096] fp32):
    energy = x_b @ x_b^T                      # (256, 256), K=4096
    attn   = softmax(rowmax(energy) - energy) # == exp(mu - e)/Z, mu = rowmin
    out    = gamma * (attn @ x_b) + x_b

Strategy (8 cores, pure data-parallel, 4 batches/core):
  - x is loaded from HBM exactly ONCE, as fp16 in t-major layout
    (xt[b, p, k, c] = x[b, c, k*128+p]) with 8KB/partition DMA lines
    (1 descriptor per partition -> cheap HWDGE issue). The c-major copy
    needed by matmul2 (xn) is derived on-chip with PE transposes.
    DMA drops from 41.9 MB/core (baseline) to 25.2 MB/core.
  - Energy is symmetric: only blocks 00/01/11 are computed; block 10 is
    a PE transpose of block 01 (saves 1/4 of matmul1).
  - matmul1 (both row-block chains) and the xn transposes are
    interleaved per k-tile so the PE tracks DMA arrival during the
    pipeline fill instead of stalling on whole tiles.
  - B = gamma*diag(1/Z)*P + I is materialized directly as the matmul2
    weights: P rows are scaled by 1/Z before the PE transpose, gamma is
    applied during the PSUM->SBUF copy of P^T, and the +x residual is
    the exact identity diagonal. matmul2 yields final output values, so
    PSUM->SBUF drains are plain copies (round-robined ACT/DVE).
  - Software pipeline: slot b = loads(b+1), mm1+xnT(b), mm2(b-1)
    (hides b's softmax latency), then softmax/At(b).
"""

import numpy as np

B, C, T = 32, 256, 4096
NCORES = 8
NB = B // NCORES  # batches per core
P = 128
KT = T // P  # 32 t-tiles of 128
KH = KT // 2  # 16 per half (xn is built as two half-tiles)
TC = T // 512  # 8 output chunks per m-block

_CACHE = {}


def _build_nc(variant=None):
    variant = variant or {}
    from contextlib import ExitStack

    import concourse.bacc as bacc
    import concourse.bass as bass
    import concourse.tile as tile
    from concourse import mybir

    f32 = mybir.dt.float32
    f16 = mybir.dt.float16
    ts = bass.ts

    nc = bacc.Bacc(
        "TRN2",
        target_bir_lowering=False,
        debug=False,
        enable_asserts=False,
        num_devices=NCORES,
    )

    # xt[b, p, k, c] = x[b, c, k*128+p]; per-partition lines are 8KB.
    xt_h = nc.dram_tensor("xt", [NB, P, KT, C], f16, kind="ExternalInput")
    # aux: col0 = gamma (f32), cols 4:132 = identity (f32)
    aux_h = nc.dram_tensor("aux", [P, 132], f32, kind="ExternalInput")
    idn_h = nc.dram_tensor("idn", [P, P], f16, kind="ExternalInput")
    o_h = nc.dram_tensor("o", [NB, C, T], f32, kind="ExternalOutput")

    with tile.TileContext(nc) as tc:
        with ExitStack() as ctx:
            singles = ctx.enter_context(tc.tile_pool(name="singles", bufs=1))
            xt_pool = ctx.enter_context(tc.tile_pool(name="xt", bufs=2))
            xn_pool = ctx.enter_context(tc.tile_pool(name="xn", bufs=3))
            out_pool = ctx.enter_context(tc.tile_pool(name="out", bufs=3))
            att_pool = ctx.enter_context(tc.tile_pool(name="att", bufs=2))
            small = ctx.enter_context(tc.tile_pool(name="small", bufs=3))
            psum_e = ctx.enter_context(
                tc.tile_pool(name="psum_e", bufs=1, space="PSUM")
            )
            psum_x = ctx.enter_context(
                tc.tile_pool(name="psum_x", bufs=2, space="PSUM")
            )
            psum_o = ctx.enter_context(
                tc.tile_pool(name="psum_o", bufs=4, space="PSUM")
            )

            xt_ap = xt_h.ap()
            o_ap = o_h.ap()

            aux = singles.tile([P, 132], f32)
            nc.scalar.dma_start(aux[:], aux_h.ap())
            idn = singles.tile([P, P], f16)
            nc.scalar.dma_start(idn[:], idn_h.ap())
            gv = aux[:, 0:1]
            idn32 = aux[:, 4:132]

            def issue_loads(b):
                # b0/b1 go through ACT (its DGE is up ~5us before SP's) in
                # strict priority order so the critical first tiles get the
                # DMA engines to themselves during the pipeline fill.
                if b == 0:
                    KQ = KT // 4
                    qs = []
                    for q in range(4):
                        t_ = xt_pool.tile(
                            [P, KQ, C], f16, tag=f"xq{q}", name=f"xq{q}"
                        )
                        nc.scalar.dma_start(
                            t_[:], xt_ap[b, :, q * KQ : (q + 1) * KQ, :]
                        )
                        qs.append(t_)
                    return (qs, KT // 4)
                eng = nc.scalar if b == 1 else nc.sync
                xta = xt_pool.tile([P, KH, C], f16, tag="xta", name="xta")
                xtb = xt_pool.tile([P, KH, C], f16, tag="xtb", name="xtb")
                eng.dma_start(xta[:], xt_ap[b, :, :KH, :])
                eng.dma_start(xtb[:], xt_ap[b, :, KH:, :])
                return ([xta, xtb], KH)

            # round-robin copy engines for PSUM->SBUF drains
            cp_engines = [
                lambda o, i: nc.scalar.copy(o, i),
                lambda o, i: nc.vector.tensor_copy(o, i),
            ]
            cp_idx = [0]

            def copy_eng():
                e = cp_engines[cp_idx[0] % len(cp_engines)]
                cp_idx[0] += 1
                return e

            def run_mm2(pb, pAt, pxn):
                """out(pb) = B^T @ x (final values) + pipelined stores."""
                # store granularity: half-tiles (2KB lines); last batch uses
                # quarter-tiles so the final store wave is only 1MB deep
                nst = 4 if pb == NB - 1 else 2
                csz = TC // nst  # 512-chunks per store tile
                TH = T // 2
                orr = o_ap[pb].rearrange("(m p) t -> p m t", p=P)
                for m in range(2):
                    for s in range(nst):
                        ot = out_pool.tile(
                            [P, csz * 512], f32, tag=f"ot{nst}", name="ot"
                        )
                        for c in range(csz):
                            t8 = s * csz + c
                            po = psum_o.tile([P, 512], f32)
                            tf = t8 * 512 % TH
                            for k in range(2):
                                nc.tensor.matmul(
                                    po[:],
                                    lhsT=pAt[m][:, k, :],
                                    rhs=pxn[t8 // 4][:, k, tf : tf + 512],
                                    start=(k == 0),
                                    stop=(k == 1),
                                )
                            copy_eng()(ot[:, ts(c, 512)], po[:])
                        nc.sync.dma_start(
                            orr[:, m, s * csz * 512 : (s + 1) * csz * 512],
                            ot[:],
                        )

            tiles = {0: issue_loads(0)}
            pending = None  # (b, At, xn) awaiting matmul2

            for b in range(NB):
                xtiles, kdiv = tiles.pop(b)
                if b + 1 < NB:
                    tiles[b + 1] = issue_loads(b + 1)

                xn = [
                    xn_pool.tile([P, 2, T // 2], f16, tag="xna", name="xna"),
                    xn_pool.tile([P, 2, T // 2], f16, tag="xnb", name="xnb"),
                ]
                At = [
                    att_pool.tile([P, 2, P], f16, tag="Ata", name="Ata"),
                    att_pool.tile([P, 2, P], f16, tag="Atb", name="Atb"),
                ]
                Zs = small.tile([P, 2], f32, tag="Zs")
                rZ = small.tile([P, 2], f32, tag="rZ")

                # ---- interleaved mm1 (blocks 00/01 + block 11) and xn
                # transposes, per k-tile, tracking DMA arrival ----
                pe1 = psum_e.tile([P, C], f32, tag="pe1", name="pe1")
                pe2 = psum_e.tile([P, C], f32, tag="pe2", name="pe2")
                for k in range(KT):
                    src = xtiles[k // kdiv]
                    kk = k % kdiv
                    nc.tensor.matmul(
                        pe1[:],
                        lhsT=src[:, kk, ts(0, P)],
                        rhs=src[:, kk, :],
                        start=(k == 0),
                        stop=(k == KT - 1),
                    )
                    nc.tensor.matmul(
                        pe2[:, ts(1, P)],
                        lhsT=src[:, kk, ts(1, P)],
                        rhs=src[:, kk, ts(1, P)],
                        start=(k == 0),
                        stop=(k == KT - 1),
                    )
                    if k % 4 == 3:
                        g4 = k - 3
                        half = g4 // KH
                        tb = (g4 % KH) * P
                        for cb in range(2):
                            px = psum_x.tile([P, 512], f16, tag="px", name="px")
                            for j in range(4):
                                kg = g4 + j
                                nc.tensor.transpose(
                                    px[:, ts(j, P)],
                                    xtiles[kg // kdiv][:, kg % kdiv, ts(cb, P)],
                                    idn[:],
                                )
                            copy_eng()(xn[half][:, cb, tb : tb + 512], px[:])

                # energy block 10 = (block 01)^T: stage 01 to SBUF, PE
                # transpose (f32) straight into pe2's first half.
                s01 = small.tile([P, P], f32, tag="s01")
                nc.scalar.copy(s01[:], pe1[:, ts(1, P)])

                # ---- previous batch's matmul2 here: hides this batch's
                # softmax/At latency behind PE work ----
                if pending is not None:
                    run_mm2(*pending)
                    pending = None

                nc.tensor.transpose(pe2[:, ts(0, P)], s01[:], idn32)

                for m in range(2):
                    pe = (pe1, pe2)[m]
                    mu = small.tile([P, 1], f32, tag="mu")
                    nc.vector.tensor_reduce(
                        mu[:], pe[:], axis=mybir.AxisListType.X,
                        op=mybir.AluOpType.min,
                    )
                    Pm = small.tile([P, C], f32, tag="Pm")
                    nc.scalar.activation(
                        Pm[:],
                        pe[:],
                        mybir.ActivationFunctionType.Exp,
                        bias=mu[:],
                        scale=-1.0,
                        accum_out=Zs[:, m : m + 1],
                    )
                    nc.vector.reciprocal(rZ[:, m : m + 1], Zs[:, m : m + 1])
                    Pm2 = small.tile([P, C], f16, tag="Pm2")
                    nc.vector.tensor_scalar_mul(Pm2[:], Pm[:], rZ[:, m : m + 1])

                    # At[m][:, k2, :] = gamma * (P/Z)^T (+ I on diagonal)
                    ptf = psum_x.tile([P, 512], f16, tag="px", name="ptf")
                    pt = ptf[:, :C]
                    for k2 in range(2):
                        nc.tensor.transpose(
                            pt[:, ts(k2, P)], Pm2[:, ts(k2, P)], idn[:]
                        )
                    nc.scalar.mul(At[m][:, :, :], pt[:], gv)
                    nc.vector.tensor_add(
                        At[m][:, m, :], At[m][:, m, :], idn[:]
                    )

                this = (b, At, xn)
                if b == NB - 1:
                    run_mm2(*this)
                else:
                    pending = this

    nc.compile()
    return nc


def _get_nc():
    if "nc" not in _CACHE:
        _CACHE["nc"] = _build_nc()
    return _CACHE["nc"]


def _make_aux(gamma_val):
    aux = np.zeros((P, 132), dtype=np.float32)
    aux[:, 0] = gamma_val
    aux[:, 4:132] = np.eye(P, dtype=np.float32)
    return aux


def kernel(x, gamma, _trace=False):
    import concourse.bass_utils as bass_utils

    x = np.ascontiguousarray(np.asarray(x, dtype=np.float32))
    gamma = np.asarray(gamma, dtype=np.float32).reshape(-1)

    nc = _get_nc()

    aux = _make_aux(gamma[0])
    idn = np.eye(P, dtype=np.float16)
    # xt[b, p, k, c] = x[b, c, k*128+p]
    xt_all = (
        x.astype(np.float16)
        .reshape(B, C, KT, P)
        .transpose(0, 3, 2, 1)
    )
    in_maps = []
    for d in range(NCORES):
        in_maps.append(
            {
                "xt": np.ascontiguousarray(xt_all[d * NB : (d + 1) * NB]),
                "aux": aux,
                "idn": idn,
            }
        )

    res = bass_utils.run_bass_kernel_spmd(
        nc, in_maps, core_ids=list(range(NCORES)), trace=_trace
    )
    out = np.concatenate([r["o"] for r in res.results], axis=0)
    if _trace:
        _CACHE["last_results"] = res
    return out


# revision 19
# speedup vs baseline: 1.2550x; 1.0410x over previous
"""Trainium2 Bass kernel for nn_Attention_Module (sparse_attention).

Computation per batch b (x_b: [C=256, T=4096] fp32):
    energy = x_b @ x_b^T                      # (256, 256), K=4096
    attn   = softmax(rowmax(energy) - energy) # == exp(mu - e)/Z, mu = rowmin
    out    = gamma * (attn @ x_b) + x_b

Strategy (8 cores, pure data-parallel, 4 batches/core):
  - x is loaded from HBM exactly ONCE, as fp16 in t-major layout
    (xt[b, p, k, c] = x[b, c, k*128+p]) with 8KB/partition DMA lines
    (1 descriptor per partition -> cheap HWDGE issue). The c-major copy
    needed by matmul2 (xn) is derived on-chip with PE transposes.
    DMA drops from 41.9 MB/core (baseline) to 25.2 MB/core.
  - Energy is symmetric: only blocks 00/01/11 are computed; block 10 is
    a PE transpose of block 01 (saves 1/4 of matmul1).
  - matmul1 (both row-block chains) and the xn transposes are
    interleaved per k-tile so the PE tracks DMA arrival during the
    pipeline fill instead of stalling on whole tiles.
  - B = gamma*diag(1/Z)*P + I is materialized directly as the matmul2
    weights: P rows are scaled by 1/Z before the PE transpose, gamma is
    applied during the PSUM->SBUF copy of P^T, and the +x residual is
    the exact identity diagonal. matmul2 yields final output values, so
    PSUM->SBUF drains are plain copies (round-robined ACT/DVE).
  - Software pipeline: slot b = loads(b+1), mm1+xnT(b), mm2(b-1)
    (hides b's softmax latency), then softmax/At(b).
"""

import numpy as np

B, C, T = 32, 256, 4096
NCORES = 8
NB = B // NCORES  # batches per core
P = 128
KT = T // P  # 32 t-tiles of 128
KH = KT // 2  # 16 per half (xn is built as two half-tiles)
TC = T // 512  # 8 output chunks per m-block

_CACHE = {}


def _build_nc(variant=None):
    variant = variant or {}
    from contextlib import ExitStack

    import concourse.bacc as bacc
    import concourse.bass as bass
    import concourse.tile as tile
    from concourse import mybir

    f32 = mybir.dt.float32
    f16 = mybir.dt.float16
    ts = bass.ts

    nc = bacc.Bacc(
        "TRN2",
        target_bir_lowering=False,
        debug=False,
        enable_asserts=False,
        num_devices=NCORES,
    )

    # xt[b, p, k, c] = x[b, c, k*128+p]; per-partition lines are 8KB.
    xt_h = nc.dram_tensor("xt", [NB, P, KT, C], f16, kind="ExternalInput")
    # xnh[b, k2, p, t'] = x[b, k2*128+p, 2048+t']: upper-t half of the
    # c-major copy, loaded directly (mid-slot, behind the critical loads)
    # so only the lower half of xn needs PE transposes.
    xnh_h = nc.dram_tensor("xnh", [NB, 2, P, T // 2], f16, kind="ExternalInput")
    # aux: col0 = gamma (f32), cols 4:132 = identity (f32)
    aux_h = nc.dram_tensor("aux", [P, 132], f32, kind="ExternalInput")
    idn_h = nc.dram_tensor("idn", [P, P], f16, kind="ExternalInput")
    o_h = nc.dram_tensor("o", [NB, C, T], f32, kind="ExternalOutput")

    with tile.TileContext(nc) as tc:
        with ExitStack() as ctx:
            singles = ctx.enter_context(tc.tile_pool(name="singles", bufs=1))
            xt_pool = ctx.enter_context(tc.tile_pool(name="xt", bufs=2))
            xn_pool = ctx.enter_context(tc.tile_pool(name="xn", bufs=3))
            out_pool = ctx.enter_context(tc.tile_pool(name="out", bufs=3))
            att_pool = ctx.enter_context(tc.tile_pool(name="att", bufs=2))
            small = ctx.enter_context(tc.tile_pool(name="small", bufs=3))
            psum_e = ctx.enter_context(
                tc.tile_pool(name="psum_e", bufs=1, space="PSUM")
            )
            psum_x = ctx.enter_context(
                tc.tile_pool(name="psum_x", bufs=2, space="PSUM")
            )
            psum_o = ctx.enter_context(
                tc.tile_pool(name="psum_o", bufs=4, space="PSUM")
            )

            xt_ap = xt_h.ap()
            xnh_ap = xnh_h.ap()
            o_ap = o_h.ap()

            aux = singles.tile([P, 132], f32)
            nc.scalar.dma_start(aux[:], aux_h.ap())
            idn = singles.tile([P, P], f16)
            nc.scalar.dma_start(idn[:], idn_h.ap())
            gv = aux[:, 0:1]
            idn32 = aux[:, 4:132]

            def issue_loads(b):
                if b == 0:
                    # four separate tiles: mm1 starts after 512KB lands
                    KQ = KT // 4
                    qs = []
                    for q in range(4):
                        t_ = xt_pool.tile(
                            [P, KQ, C], f16, tag=f"xq{q}", name=f"xq{q}"
                        )
                        nc.sync.dma_start(
                            t_[:], xt_ap[b, :, q * KQ : (q + 1) * KQ, :]
                        )
                        qs.append(t_)
                    return (qs, KT // 4)
                xta = xt_pool.tile([P, KH, C], f16, tag="xta", name="xta")
                xtb = xt_pool.tile([P, KH, C], f16, tag="xtb", name="xtb")
                nc.sync.dma_start(xta[:], xt_ap[b, :, :KH, :])
                nc.sync.dma_start(xtb[:], xt_ap[b, :, KH:, :])
                return ([xta, xtb], KH)

            # round-robin copy engines for PSUM->SBUF drains
            cp_engines = [
                lambda o, i: nc.scalar.copy(o, i),
                lambda o, i: nc.vector.tensor_copy(o, i),
            ]
            cp_idx = [0]

            def copy_eng():
                e = cp_engines[cp_idx[0] % len(cp_engines)]
                cp_idx[0] += 1
                return e

            def run_mm2(pb, pAt, pxn):
                """out(pb) = B^T @ x (final values) + stores."""
                nsplit = 4 if pb == NB - 1 else 2
                TH = T // 2
                for m in range(2):
                    ot = out_pool.tile([P, T], f32, tag="ot", name="ot")
                    for t8 in range(TC):  # [P,512] chunks, 4-deep psum ring
                        po = psum_o.tile([P, 512], f32)
                        tf = t8 * 512 % TH
                        for k in range(2):
                            nc.tensor.matmul(
                                po[:],
                                lhsT=pAt[m][:, k, :],
                                rhs=pxn[t8 // 4][:, k, tf : tf + 512],
                                start=(k == 0),
                                stop=(k == 1),
                            )
                        copy_eng()(ot[:, ts(t8, 512)], po[:])
                    for sh in range(nsplit):
                        nc.sync.dma_start(
                            o_ap[pb].rearrange("(m p) t -> p m t", p=P)[
                                :, m, ts(sh, T // nsplit)
                            ],
                            ot[:, ts(sh, T // nsplit)],
                        )

            tiles = {0: issue_loads(0)}
            pending = None  # (b, At, xn) awaiting matmul2

            for b in range(NB):
                xtiles, kdiv = tiles.pop(b)
                if b + 1 < NB:
                    tiles[b + 1] = issue_loads(b + 1)

                xn = [
                    xn_pool.tile([P, 2, T // 2], f16, tag="xna", name="xna"),
                    xn_pool.tile([P, 2, T // 2], f16, tag="xnb", name="xnb"),
                ]
                nc.sync.dma_start(
                    xn[1][:], xnh_ap[b].rearrange("k p t -> p k t")
                )
                At = [
                    att_pool.tile([P, 2, P], f16, tag="Ata", name="Ata"),
                    att_pool.tile([P, 2, P], f16, tag="Atb", name="Atb"),
                ]
                Zs = small.tile([P, 2], f32, tag="Zs")
                rZ = small.tile([P, 2], f32, tag="rZ")

                # ---- interleaved mm1 (blocks 00/01 + block 11) and xn
                # transposes, per k-tile, tracking DMA arrival ----
                pe1 = psum_e.tile([P, C], f32, tag="pe1", name="pe1")
                pe2 = psum_e.tile([P, C], f32, tag="pe2", name="pe2")
                for k in range(KT):
                    src = xtiles[k // kdiv]
                    kk = k % kdiv
                    nc.tensor.matmul(
                        pe1[:],
                        lhsT=src[:, kk, ts(0, P)],
                        rhs=src[:, kk, :],
                        start=(k == 0),
                        stop=(k == KT - 1),
                    )
                    nc.tensor.matmul(
                        pe2[:, ts(1, P)],
                        lhsT=src[:, kk, ts(1, P)],
                        rhs=src[:, kk, ts(1, P)],
                        start=(k == 0),
                        stop=(k == KT - 1),
                    )
                    if k % 4 == 3 and k < KH:
                        g4 = k - 3
                        half = 0
                        tb = g4 * P
                        for cb in range(2):
                            px = psum_x.tile([P, 512], f16, tag="px", name="px")
                            for j in range(4):
                                kg = g4 + j
                                nc.tensor.transpose(
                                    px[:, ts(j, P)],
                                    xtiles[kg // kdiv][:, kg % kdiv, ts(cb, P)],
                                    idn[:],
                                )
                            copy_eng()(xn[half][:, cb, tb : tb + 512], px[:])

                # energy block 10 = (block 01)^T: stage 01 to SBUF, PE
                # transpose (f32) straight into pe2's first half.
                s01 = small.tile([P, P], f32, tag="s01")
                nc.scalar.copy(s01[:], pe1[:, ts(1, P)])

                # ---- previous batch's matmul2 here: hides this batch's
                # softmax/At latency behind PE work ----
                if pending is not None:
                    run_mm2(*pending)
                    pending = None

                nc.tensor.transpose(pe2[:, ts(0, P)], s01[:], idn32)

                for m in range(2):
                    pe = (pe1, pe2)[m]
                    mu = small.tile([P, 1], f32, tag="mu")
                    nc.vector.tensor_reduce(
                        mu[:], pe[:], axis=mybir.AxisListType.X,
                        op=mybir.AluOpType.min,
                    )
                    Pm = small.tile([P, C], f32, tag="Pm")
                    nc.scalar.activation(
                        Pm[:],
                        pe[:],
                        mybir.ActivationFunctionType.Exp,
                        bias=mu[:],
                        scale=-1.0,
                        accum_out=Zs[:, m : m + 1],
                    )
                    nc.vector.reciprocal(rZ[:, m : m + 1], Zs[:, m : m + 1])
                    Pm2 = small.tile([P, C], f16, tag="Pm2")
                    nc.vector.tensor_scalar_mul(Pm2[:], Pm[:], rZ[:, m : m + 1])

                    # At[m][:, k2, :] = gamma * (P/Z)^T (+ I on diagonal)
                    ptf = psum_x.tile([P, 512], f16, tag="px", name="ptf")
                    pt = ptf[:, :C]
                    for k2 in range(2):
                        nc.tensor.transpose(
                            pt[:, ts(k2, P)], Pm2[:, ts(k2, P)], idn[:]
                        )
                    nc.scalar.mul(At[m][:, :, :], pt[:], gv)
                    nc.vector.tensor_add(
                        At[m][:, m, :], At[m][:, m, :], idn[:]
                    )

                this = (b, At, xn)
                if b == NB - 1:
                    run_mm2(*this)
                else:
                    pending = this

    nc.compile()
    return nc


def _get_nc():
    if "nc" not in _CACHE:
        _CACHE["nc"] = _build_nc()
    return _CACHE["nc"]


def _make_aux(gamma_val):
    aux = np.zeros((P, 132), dtype=np.float32)
    aux[:, 0] = gamma_val
    aux[:, 4:132] = np.eye(P, dtype=np.float32)
    return aux


def kernel(x, gamma, _trace=False):
    import concourse.bass_utils as bass_utils

    x = np.ascontiguousarray(np.asarray(x, dtype=np.float32))
    gamma = np.asarray(gamma, dtype=np.float32).reshape(-1)

    nc = _get_nc()

    aux = _make_aux(gamma[0])
    idn = np.eye(P, dtype=np.float16)
    # xt[b, p, k, c] = x[b, c, k*128+p]
    xt_all = (
        x.astype(np.float16)
        .reshape(B, C, KT, P)
        .transpose(0, 3, 2, 1)
    )
    xnh_all = (
        x[:, :, T // 2 :].astype(np.float16).reshape(B, 2, P, T // 2)
    )
    in_maps = []
    for d in range(NCORES):
        in_maps.append(
            {
                "xt": np.ascontiguousarray(xt_all[d * NB : (d + 1) * NB]),
                "xnh": np.ascontiguousarray(xnh_all[d * NB : (d + 1) * NB]),
                "aux": aux,
                "idn": idn,
            }
        )

    res = bass_utils.run_bass_kernel_spmd(
        nc, in_maps, core_ids=list(range(NCORES)), trace=_trace
    )
    out = np.concatenate([r["o"] for r in res.results], axis=0)
    if _trace:
        _CACHE["last_results"] = res
    return out
